# revision 10
# baseline (speedup 1.0000x reference)
"""Bass/Tile kernel for batched cross-attention on 8 TRN2 NeuronCores.

Problem (per reference):
    encoder_output: [S=2048, B=8, H=1024] f32
    decoder_hidden: [T=2048, B=8, H=1024] f32
    energies[b,t,s] = dec[t,b,:] . enc[s,b,:]
    weights = softmax(energies, axis=s)           -> returned as [T, B, S]
    context[t,b,:] = sum_s weights[b,t,s]*enc[s,b,:]  -> [T, B, H]

Sharding: batch b -> core b (pure data parallel, no collectives).

Per-core pipeline (T tiles of 128 rows):
    MM1 (TensorE, bf16):  E = dec_b @ enc_b^T     [128, S] fp32 in PSUM
    VectorE:              rowmax -> negmax
    ScalarE:              w = exp(E - max) (bf16) + accumulated rowsum
    DMA xbar:             w^T tiles for MM2
    TensorE (bf16):       C = w @ enc_b           [128, H] fp32 in PSUM
    ScalarE:              weights_out = w * (1/sum)  (f32), ctx_out = C * (1/sum)
The t-tile loop is software-pipelined (MM1 of tile i+1 is issued before
MM2 of tile i-1) so the PE stream never waits on the softmax/transpose.
"""

import os
import sys

for _p in ("/opt/trn_rl_repo", "/root/.axon_site/_ro/trn_rl_repo"):
    if os.path.isdir(_p) and _p not in sys.path:
        sys.path.insert(0, _p)

from contextlib import ExitStack

import ml_dtypes
import numpy as np

import concourse.bass as bass
import concourse.tile as tile
from concourse import bacc, mybir
from concourse.bass_utils import run_bass_kernel_spmd

S, T, B, H = 2048, 2048, 8, 1024
P = 128  # SBUF partitions
NBANK = 512  # fp32 PSUM bank free size / max moving free dim

BF16 = mybir.dt.bfloat16
F32 = mybir.dt.float32
AX_X = mybir.AxisListType.X
OP_MAX = mybir.AluOpType.max
OP_ADD = mybir.AluOpType.add
ACT_EXP = mybir.ActivationFunctionType.Exp
ACT_COPY = mybir.ActivationFunctionType.Copy


def build_kernel(t_dim=T, s_dim=S, h_dim=H):
    """Build and compile the per-core Bass module (same program on all cores)."""
    assert t_dim % P == 0 and s_dim % NBANK == 0 and h_dim % NBANK == 0

    kh = h_dim // P  # contraction chunks for MM1
    ks = s_dim // P  # contraction chunks for MM2
    nt = t_dim // P  # t tiles
    ns = s_dim // NBANK  # PSUM banks for energies
    nh = h_dim // NBANK  # PSUM-bank halves for context

    nc = bacc.Bacc("TRN2", target_bir_lowering=False, debug=False, num_devices=8)

    encT_d = nc.dram_tensor("encT", [h_dim, s_dim], BF16, kind="ExternalInput").ap()
    # decC[i, p, k*P+t] = dec[i*P+t, k*P+p]: per-t-tile chunks of dec^T, host-packed
    decC_d = nc.dram_tensor(
        "decC", [t_dim // P, P, h_dim], BF16, kind="ExternalInput"
    ).ap()
    enc_d = nc.dram_tensor("enc", [s_dim, h_dim], BF16, kind="ExternalInput").ap()
    wout_d = nc.dram_tensor("weights", [t_dim, s_dim], F32, kind="ExternalOutput").ap()
    cout_d = nc.dram_tensor("context", [t_dim, h_dim], F32, kind="ExternalOutput").ap()

    with tile.TileContext(nc) as tc, ExitStack() as ctx:
        resident = ctx.enter_context(tc.tile_pool(name="resident", bufs=1))
        epool = ctx.enter_context(tc.tile_pool(name="epool", bufs=1, space="PSUM"))
        cpool = ctx.enter_context(tc.tile_pool(name="cpool", bufs=1, space="PSUM"))
        wbf_pool = ctx.enter_context(tc.tile_pool(name="wbf_pool", bufs=2))
        wf32_pool = ctx.enter_context(tc.tile_pool(name="wf32_pool", bufs=2))
        wt_pool = ctx.enter_context(tc.tile_pool(name="wt_pool", bufs=2))
        cs_pool = ctx.enter_context(tc.tile_pool(name="cs_pool", bufs=2))
        small = ctx.enter_context(tc.tile_pool(name="small", bufs=4))

        # Load order matters for the prologue: MM1(0) only needs encT[0] +
        # dec_cols[0]; stream the rest while the PE is already running.
        encT_sb = [None] * kh
        dec_cols = [None] * nt  # [P, kh, P]: dec_cols[i][:, k, :] = decT[kP:(k+1)P, iP:(i+1)P]
        enc_sb = [None] * ks

        def load_encT(k):
            t_ = resident.tile([P, s_dim], BF16, name=f"encT_sb{k}", tag=f"encT{k}")
            nc.sync.dma_start(t_[:], encT_d[k * P : (k + 1) * P, :])
            encT_sb[k] = t_

        def load_dec_cols(i):
            t_ = resident.tile([P, h_dim], BF16, name=f"dec_cols{i}", tag=f"dec_cols{i}")
            nc.sync.dma_start(t_[:], decC_d[i, :, :])
            dec_cols[i] = t_

        def load_enc(j):
            t_ = resident.tile([P, h_dim], BF16, name=f"enc_sb{j}", tag=f"enc{j}")
            nc.scalar.dma_start(t_[:], enc_d[j * P : (j + 1) * P, :])
            enc_sb[j] = t_

        load_encT(0)
        load_dec_cols(0)
        for k in range(1, kh):
            load_encT(k)
        for i in range(1, nt):
            load_dec_cols(i)
        for j in range(ks):
            load_enc(j)

        state = {}

        def mm1(i):
            t0 = i * P
            eb = [
                epool.tile([P, NBANK], F32, name=f"eb{i}_{n}", tag=f"e{n}")
                for n in range(ns)
            ]
            for k in range(kh):
                lhsT = dec_cols[i][:, k * P : (k + 1) * P]
                for n in range(ns):
                    nc.tensor.matmul(
                        eb[n][:],
                        lhsT,
                        encT_sb[k][:, n * NBANK : (n + 1) * NBANK],
                        start=(k == 0),
                        stop=(k == kh - 1),
                    )
            state[i] = {"eb": eb}

        def softmax(i):
            st = state[i]
            eb = st["eb"]
            maxs = small.tile([P, ns], F32, name=f"maxs{i}", tag="maxs")
            for n in range(ns):
                nc.vector.tensor_reduce(maxs[:, n : n + 1], eb[n][:], AX_X, OP_MAX)
            negmax = small.tile([P, 1], F32, name=f"negmax{i}", tag="negmax")
            nc.vector.tensor_reduce(negmax[:], maxs[:], AX_X, OP_MAX, negate=True)

            wbf = wbf_pool.tile([P, s_dim], BF16, name=f"wbf{i}", tag="wbf")
            sums = small.tile([P, ns], F32, name=f"sums{i}", tag="sums")
            for n in range(ns):
                nc.scalar.activation(
                    wbf[:, n * NBANK : (n + 1) * NBANK],
                    eb[n][:],
                    ACT_EXP,
                    bias=negmax[:, 0:1],
                    accum_out=sums[:, n : n + 1],
                )
            ssum = small.tile([P, 1], F32, name=f"ssum{i}", tag="ssum")
            nc.vector.tensor_reduce(ssum[:], sums[:], AX_X, OP_ADD)
            r = small.tile([P, 1], F32, name=f"r{i}", tag="r")
            nc.vector.reciprocal(r[:], ssum[:])

            # normalized fp32 attention weights -> DRAM
            wf = wf32_pool.tile([P, s_dim], F32, name=f"wf{i}", tag="wf")
            nc.scalar.activation(wf[:], wbf[:], ACT_COPY, scale=r[:, 0:1])
            nc.scalar.dma_start(wout_d[i * P : (i + 1) * P, :], wf[:])

            # transposed bf16 weights for MM2 (DMA xbar transpose, SBUF->SBUF).
            # One call: out[:, j, :] == transpose(wbf[:, j*P:(j+1)*P]) (verified).
            wt = wt_pool.tile([P, ks, P], BF16, name=f"wt{i}", tag="wt")
            nc.sync.dma_start(wt[:], wbf[:], transpose=True)
            st["wt"] = wt
            st["r"] = r

        def mm2(i):
            st = state.pop(i)
            wt = st["wt"]
            cb = cpool.tile([P, h_dim], F32, name=f"cb{i}", tag="cb")
            for j in range(ks):
                lhsT = wt[:, j, :]
                for n in range(nh):
                    nc.tensor.matmul(
                        cb[:, n * NBANK : (n + 1) * NBANK],
                        lhsT,
                        enc_sb[j][:, n * NBANK : (n + 1) * NBANK],
                        start=(j == 0),
                        stop=(j == ks - 1),
                    )
            cs = cs_pool.tile([P, h_dim], F32, name=f"cs{i}", tag="cs")
            nc.scalar.activation(cs[:], cb[:], ACT_COPY, scale=st["r"][:, 0:1])
            nc.scalar.dma_start(cout_d[i * P : (i + 1) * P, :], cs[:])

        # software pipeline: PE stream = MM1(0) MM1(1) [MM1(i+1) MM2(i-1)]...
        mm1(0)
        softmax(0)
        if nt > 1:
            mm1(1)
        for i in range(1, nt):
            softmax(i)
            if i + 1 < nt:
                mm1(i + 1)
            mm2(i - 1)
        mm2(nt - 1)

    nc.compile()
    return nc


_NC_CACHE = {}


def _get_nc(shape_key):
    if shape_key not in _NC_CACHE:
        _NC_CACHE[shape_key] = build_kernel(*shape_key)
    return _NC_CACHE[shape_key]


def kernel(encoder_output, decoder_hidden, _trace=False, _tmpdir=None):
    encoder_output = np.asarray(encoder_output)
    decoder_hidden = np.asarray(decoder_hidden)
    s_dim, b_dim, h_dim = encoder_output.shape
    t_dim = decoder_hidden.shape[0]

    nc = _get_nc((t_dim, s_dim, h_dim))

    bf = ml_dtypes.bfloat16
    in_maps = []
    for b in range(b_dim):
        enc_b = np.ascontiguousarray(encoder_output[:, b, :]).astype(bf)
        dec_b = np.ascontiguousarray(decoder_hidden[:, b, :]).astype(bf)
        nt, kh = t_dim // P, h_dim // P
        decC = np.ascontiguousarray(
            dec_b.reshape(nt, P, kh, P).transpose(0, 3, 2, 1)
        ).reshape(nt, P, h_dim)
        in_maps.append(
            {
                "encT": np.ascontiguousarray(enc_b.T),
                "decC": decC,
                "enc": enc_b,
            }
        )

    res = run_bass_kernel_spmd(
        nc, in_maps, core_ids=list(range(b_dim)), trace=_trace, tmpdir=_tmpdir
    )
    kernel.last_results = res

    context = np.empty((t_dim, b_dim, h_dim), dtype=np.float32)
    weights = np.empty((t_dim, b_dim, s_dim), dtype=np.float32)
    for b in range(b_dim):
        context[:, b, :] = res.results[b]["context"]
        weights[:, b, :] = res.results[b]["weights"]

    kernel.last_exec_time_ns = res.exec_time_ns
    return (context, weights)


# revision 12
# speedup vs baseline: 1.0097x; 1.0097x over previous
"""Bass/Tile kernel for batched cross-attention on 8 TRN2 NeuronCores.

Problem (per reference):
    encoder_output: [S=2048, B=8, H=1024] f32
    decoder_hidden: [T=2048, B=8, H=1024] f32
    energies[b,t,s] = dec[t,b,:] . enc[s,b,:]
    weights = softmax(energies, axis=s)           -> returned as [T, B, S]
    context[t,b,:] = sum_s weights[b,t,s]*enc[s,b,:]  -> [T, B, H]

Sharding: batch b -> core b (pure data parallel, no collectives).

Per-core pipeline (T tiles of 128 rows):
    MM1 (TensorE, bf16):  E = dec_b @ enc_b^T     [128, S] fp32 in PSUM
    VectorE:              rowmax -> negmax
    ScalarE:              w = exp(E - max) (bf16) + accumulated rowsum
    DMA xbar:             w^T tiles for MM2
    TensorE (bf16):       C = w @ enc_b           [128, H] fp32 in PSUM
    ScalarE:              weights_out = w * (1/sum)  (f32), ctx_out = C * (1/sum)
The t-tile loop is software-pipelined (MM1 of tile i+1 is issued before
MM2 of tile i-1) so the PE stream never waits on the softmax/transpose.
"""

import os
import sys

for _p in ("/opt/trn_rl_repo", "/root/.axon_site/_ro/trn_rl_repo"):
    if os.path.isdir(_p) and _p not in sys.path:
        sys.path.insert(0, _p)

from contextlib import ExitStack

import ml_dtypes
import numpy as np

import concourse.bass as bass
import concourse.tile as tile
from concourse import bacc, mybir
from concourse.bass_utils import run_bass_kernel_spmd

S, T, B, H = 2048, 2048, 8, 1024
P = 128  # SBUF partitions
NBANK = 512  # fp32 PSUM bank free size / max moving free dim

BF16 = mybir.dt.float16  # fp16: same PE rate as bf16, 8x the mantissa for unit-scale data
F32 = mybir.dt.float32
AX_X = mybir.AxisListType.X
OP_MAX = mybir.AluOpType.max
OP_ADD = mybir.AluOpType.add
ACT_EXP = mybir.ActivationFunctionType.Exp
ACT_COPY = mybir.ActivationFunctionType.Copy


def build_kernel(t_dim=T, s_dim=S, h_dim=H):
    """Build and compile the per-core Bass module (same program on all cores)."""
    assert t_dim % P == 0 and s_dim % NBANK == 0 and h_dim % NBANK == 0

    kh = h_dim // P  # contraction chunks for MM1
    ks = s_dim // P  # contraction chunks for MM2
    nt = t_dim // P  # t tiles
    ns = s_dim // NBANK  # PSUM banks for energies
    nh = h_dim // NBANK  # PSUM-bank halves for context

    nc = bacc.Bacc("TRN2", target_bir_lowering=False, debug=False, num_devices=8)

    encT_d = nc.dram_tensor("encT", [h_dim, s_dim], BF16, kind="ExternalInput").ap()
    # decC[i, p, k*P+t] = dec[i*P+t, k*P+p]: per-t-tile chunks of dec^T, host-packed
    decC_d = nc.dram_tensor(
        "decC", [t_dim // P, P, h_dim], BF16, kind="ExternalInput"
    ).ap()
    enc_d = nc.dram_tensor("enc", [s_dim, h_dim], BF16, kind="ExternalInput").ap()
    wout_d = nc.dram_tensor("weights", [t_dim, s_dim], F32, kind="ExternalOutput").ap()
    cout_d = nc.dram_tensor("context", [t_dim, h_dim], F32, kind="ExternalOutput").ap()

    with tile.TileContext(nc) as tc, ExitStack() as ctx:
        resident = ctx.enter_context(tc.tile_pool(name="resident", bufs=1))
        epool = ctx.enter_context(tc.tile_pool(name="epool", bufs=1, space="PSUM"))
        cpool = ctx.enter_context(tc.tile_pool(name="cpool", bufs=1, space="PSUM"))
        wbf_pool = ctx.enter_context(tc.tile_pool(name="wbf_pool", bufs=2))
        wf32_pool = ctx.enter_context(tc.tile_pool(name="wf32_pool", bufs=2))
        wt_pool = ctx.enter_context(tc.tile_pool(name="wt_pool", bufs=2))
        cs_pool = ctx.enter_context(tc.tile_pool(name="cs_pool", bufs=2))
        small = ctx.enter_context(tc.tile_pool(name="small", bufs=4))

        # Load order matters for the prologue: MM1(0) only needs encT[0] +
        # dec_cols[0]; stream the rest while the PE is already running.
        encT_sb = [None] * kh
        dec_cols = [None] * nt  # [P, kh, P]: dec_cols[i][:, k, :] = decT[kP:(k+1)P, iP:(i+1)P]
        enc_sb = [None] * ks

        def load_encT(k):
            t_ = resident.tile([P, s_dim], BF16, name=f"encT_sb{k}", tag=f"encT{k}")
            nc.sync.dma_start(t_[:], encT_d[k * P : (k + 1) * P, :])
            encT_sb[k] = t_

        def load_dec_cols(i):
            t_ = resident.tile([P, h_dim], BF16, name=f"dec_cols{i}", tag=f"dec_cols{i}")
            nc.sync.dma_start(t_[:], decC_d[i, :, :])
            dec_cols[i] = t_

        def load_enc(j):
            t_ = resident.tile([P, h_dim], BF16, name=f"enc_sb{j}", tag=f"enc{j}")
            nc.scalar.dma_start(t_[:], enc_d[j * P : (j + 1) * P, :])
            enc_sb[j] = t_

        load_encT(0)
        load_dec_cols(0)
        for k in range(1, kh):
            load_encT(k)
        for i in range(1, nt):
            load_dec_cols(i)
        for j in range(ks):
            load_enc(j)

        # Warm the PE HAM clock gate during the DMA prologue: ~28 dummy
        # matmuls on a zeroed scratch tile keep the array busy >3.4us so the
        # real stream starts at 2.4 GHz instead of 1.2.
        wpool = ctx.enter_context(tc.tile_pool(name="wpool", bufs=1))
        wpsum = ctx.enter_context(tc.tile_pool(name="wpsum", bufs=1, space="PSUM"))
        scratch = wpool.tile([P, NBANK], BF16, name="warm_src", tag="warm_src")
        nc.vector.memset(scratch[:], 0.0)
        warm_ps = wpsum.tile([P, NBANK], F32, name="warm_ps", tag="warm_ps")
        for _ in range(28):
            nc.tensor.matmul(
                warm_ps[:], scratch[:, 0:P], scratch[:], start=True, stop=True
            )

        state = {}

        def mm1(i):
            t0 = i * P
            eb = [
                epool.tile([P, NBANK], F32, name=f"eb{i}_{n}", tag=f"e{n}")
                for n in range(ns)
            ]
            for k in range(kh):
                lhsT = dec_cols[i][:, k * P : (k + 1) * P]
                for n in range(ns):
                    nc.tensor.matmul(
                        eb[n][:],
                        lhsT,
                        encT_sb[k][:, n * NBANK : (n + 1) * NBANK],
                        start=(k == 0),
                        stop=(k == kh - 1),
                    )
            state[i] = {"eb": eb}

        def softmax(i):
            st = state[i]
            eb = st["eb"]
            maxs = small.tile([P, ns], F32, name=f"maxs{i}", tag="maxs")
            for n in range(ns):
                nc.vector.tensor_reduce(maxs[:, n : n + 1], eb[n][:], AX_X, OP_MAX)
            negmax = small.tile([P, 1], F32, name=f"negmax{i}", tag="negmax")
            nc.vector.tensor_reduce(negmax[:], maxs[:], AX_X, OP_MAX, negate=True)

            wbf = wbf_pool.tile([P, s_dim], BF16, name=f"wbf{i}", tag="wbf")
            sums = small.tile([P, ns], F32, name=f"sums{i}", tag="sums")
            for n in range(ns):
                nc.scalar.activation(
                    wbf[:, n * NBANK : (n + 1) * NBANK],
                    eb[n][:],
                    ACT_EXP,
                    bias=negmax[:, 0:1],
                    accum_out=sums[:, n : n + 1],
                )
            ssum = small.tile([P, 1], F32, name=f"ssum{i}", tag="ssum")
            nc.vector.tensor_reduce(ssum[:], sums[:], AX_X, OP_ADD)
            r = small.tile([P, 1], F32, name=f"r{i}", tag="r")
            nc.vector.reciprocal(r[:], ssum[:])

            # normalized fp32 attention weights -> DRAM
            wf = wf32_pool.tile([P, s_dim], F32, name=f"wf{i}", tag="wf")
            nc.scalar.activation(wf[:], wbf[:], ACT_COPY, scale=r[:, 0:1])
            nc.scalar.dma_start(wout_d[i * P : (i + 1) * P, :], wf[:])

            # transposed bf16 weights for MM2 (DMA xbar transpose, SBUF->SBUF).
            # One call: out[:, j, :] == transpose(wbf[:, j*P:(j+1)*P]) (verified).
            wt = wt_pool.tile([P, ks, P], BF16, name=f"wt{i}", tag="wt")
            nc.sync.dma_start(wt[:], wbf[:], transpose=True)
            st["wt"] = wt
            st["r"] = r

        def mm2(i):
            st = state.pop(i)
            wt = st["wt"]
            cb = cpool.tile([P, h_dim], F32, name=f"cb{i}", tag="cb")
            for j in range(ks):
                lhsT = wt[:, j, :]
                for n in range(nh):
                    nc.tensor.matmul(
                        cb[:, n * NBANK : (n + 1) * NBANK],
                        lhsT,
                        enc_sb[j][:, n * NBANK : (n + 1) * NBANK],
                        start=(j == 0),
                        stop=(j == ks - 1),
                    )
            cs = cs_pool.tile([P, h_dim], F32, name=f"cs{i}", tag="cs")
            nc.scalar.activation(cs[:], cb[:], ACT_COPY, scale=st["r"][:, 0:1])
            nc.scalar.dma_start(cout_d[i * P : (i + 1) * P, :], cs[:])

        # software pipeline: PE stream = MM1(0) MM1(1) [MM1(i+1) MM2(i-1)]...
        mm1(0)
        softmax(0)
        if nt > 1:
            mm1(1)
        for i in range(1, nt):
            softmax(i)
            if i + 1 < nt:
                mm1(i + 1)
            mm2(i - 1)
        mm2(nt - 1)

    nc.compile()
    return nc


_NC_CACHE = {}


def _get_nc(shape_key):
    if shape_key not in _NC_CACHE:
        _NC_CACHE[shape_key] = build_kernel(*shape_key)
    return _NC_CACHE[shape_key]


def kernel(encoder_output, decoder_hidden, _trace=False, _tmpdir=None):
    encoder_output = np.asarray(encoder_output)
    decoder_hidden = np.asarray(decoder_hidden)
    s_dim, b_dim, h_dim = encoder_output.shape
    t_dim = decoder_hidden.shape[0]

    nc = _get_nc((t_dim, s_dim, h_dim))

    bf = np.float16
    in_maps = []
    for b in range(b_dim):
        enc_b = np.ascontiguousarray(encoder_output[:, b, :]).astype(bf)
        dec_b = np.ascontiguousarray(decoder_hidden[:, b, :]).astype(bf)
        nt, kh = t_dim // P, h_dim // P
        decC = np.ascontiguousarray(
            dec_b.reshape(nt, P, kh, P).transpose(0, 3, 2, 1)
        ).reshape(nt, P, h_dim)
        in_maps.append(
            {
                "encT": np.ascontiguousarray(enc_b.T),
                "decC": decC,
                "enc": enc_b,
            }
        )

    res = run_bass_kernel_spmd(
        nc, in_maps, core_ids=list(range(b_dim)), trace=_trace, tmpdir=_tmpdir
    )
    kernel.last_results = res

    context = np.empty((t_dim, b_dim, h_dim), dtype=np.float32)
    weights = np.empty((t_dim, b_dim, s_dim), dtype=np.float32)
    for b in range(b_dim):
        context[:, b, :] = res.results[b]["context"]
        weights[:, b, :] = res.results[b]["weights"]

    kernel.last_exec_time_ns = res.exec_time_ns
    return (context, weights)
